# revision 16
# baseline (speedup 1.0000x reference)
"""Masked causal self-attention on 8 trn2 NeuronCores.

Problem: x[4,4096,1024] fp32; q/k/v = x @ W{q,k,v}.T (D=64);
out = softmax(causal(q k^T / 8)) v   -> [4, 4096, 64].

Sharding: core = (batch, parity). Each core loads its batch's full x,
builds k/v for all 4096 rows, and computes attention for the 2048 q rows
it owns (alternating 128-row blocks by parity). SPMD requires one
program for all cores, so per-core differences are carried by data only:
  - parity-1 cores receive x with adjacent 128-row blocks swapped, so
    every core's own q-blocks sit at even block positions;
  - the causal masks (which differ under that permutation) are inputs.

On-chip dataflow per core (all matmuls float32r = full PE rate):
  x [rows,E] --PE transpose--> xT [E,rows] --matmul--> kT/vT/qT
  scores are computed transposed: S^T[kv,q] = kT-block.T @ qT
  softmax without max-subtraction (scores ~ N(0,1), exp is safe in fp32),
  masked after exp by multiplying with 0/1 mask tiles; the softmax
  denominators come free from an appended ones-column in the V stationary
  ([v | 1] -> row 64 of the output accumulator is sum(exp)).
  oT accumulates in PSUM over kv blocks, is normalized, transposed back,
  and DMA'd out.
"""

import sys

sys.path.insert(0, "/opt/trn_rl_repo")

import numpy as np

B, S, E, D = 4, 4096, 1024, 64
P = 128
NBLK = S // P            # 32 kv block positions
NITER = 8                # phase-1 iterations, 512 rows each
NSUP = 4                 # phase-2 q superblocks, 512 own q rows each
OWN = S // 2             # own q rows per core

_prog_cache = {}


def _build_program():
    import concourse.mybir as mybir
    from concourse import bacc, tile

    f32r = mybir.dt.float32r
    f32 = mybir.dt.float32
    bf16 = mybir.dt.bfloat16

    nc = bacc.Bacc("TRN2", target_bir_lowering=False, debug=False, num_devices=8)
    x_d = nc.dram_tensor("x", [S, E], f32r, kind="ExternalInput")
    wkv_d = nc.dram_tensor("wkv", [P, 8 * 128], bf16, kind="ExternalInput")
    wq_d = nc.dram_tensor("wq", [P, 8 * 64], bf16, kind="ExternalInput")
    mask_d = nc.dram_tensor("mask", [P, 8 * 128], bf16, kind="ExternalInput")
    ident_d = nc.dram_tensor("ident", [P, P], f32r, kind="ExternalInput")
    identb_d = nc.dram_tensor("identb", [P, P], bf16, kind="ExternalInput")
    ones_d = nc.dram_tensor("ones", [P, NBLK], bf16, kind="ExternalInput")
    y_d = nc.dram_tensor("y", [OWN, D], f32r, kind="ExternalOutput")

    with tile.TileContext(nc) as tc:
        with (
            tc.tile_pool(name="const", bufs=1) as constp,
            tc.tile_pool(name="xin", bufs=3) as xin,
            tc.tile_pool(name="xt", bufs=2) as xtp,
            tc.tile_pool(name="work", bufs=3) as work,
            tc.tile_pool(name="ps_big", bufs=2, space="PSUM") as ps_big,
            tc.tile_pool(name="ps_kv", bufs=1, space="PSUM") as ps_kv,
            tc.tile_pool(name="ps_small", bufs=1, space="PSUM") as ps_small,
                        tc.tile_pool(name="ps_o", bufs=2, space="PSUM") as ps_o,
        ):
            # ---- persistent state; constant DMAs are emitted inside the
            # driver loop after the first x prefetch (sync queue) or routed
            # through the idle gpsimd queue ----
            ident = constp.tile([P, P], f32r, tag="ident")
            identb = constp.tile([P, P], bf16, tag="identb")
            wkv_sb = constp.tile([P, 8, 128], bf16, tag="wkv")
            wq_sb = constp.tile([P, 8, 64], bf16, tag="wq")
            mask_sb = constp.tile([P, 8, 128], bf16, tag="mask")
            kT_sb = constp.tile([64, S], bf16, tag="kT")
            qT_sb = constp.tile([64, OWN], bf16, tag="qT")
            vOnes = constp.tile([P, NBLK, 65], bf16, tag="vOnes")

            def load_consts():
                nc.sync.dma_start(identb[:], identb_d.ap())
                nc.sync.dma_start(
                    wkv_sb[:], wkv_d.ap().rearrange("p (c m) -> p c m", c=8)
                )
                nc.sync.dma_start(
                    wq_sb[:], wq_d.ap().rearrange("p (c m) -> p c m", c=8)
                )
                nc.gpsimd.dma_start(ident[:], ident_d.ap())
                nc.gpsimd.dma_start(
                    mask_sb[:], mask_d.ap().rearrange("p (k c) -> p k c", k=8)
                )
                nc.gpsimd.dma_start(vOnes[:, :, 64], ones_d.ap())

            # ---- phase 1: prefetch (DMA) and compute bodies ----
            x_tiles = {}

            def prefetch_x(it):
                r0 = it * 512
                blks = []
                for i in range(4):
                    xn = xin.tile([P, E], f32r, tag=f"xnat{i}", name=f"xnat_{it}_{i}")
                    nc.sync.dma_start(
                        xn[:],
                        x_d.ap()[r0 + i * P : r0 + (i + 1) * P].rearrange(
                            "(i p) e -> p (i e)", i=1
                        ),
                    )
                    blks.append(xn)
                x_tiles[it] = blks

            def phase1_iter(it):
                r0 = it * 512
                x_nat = x_tiles.pop(it)
                x_bf = [
                    xin.tile([P, E], bf16, tag=f"xbf{i}", name=f"xbf_{it}_{i}")
                    for i in range(4)
                ]
                for i in range(4):
                    nc.gpsimd.tensor_copy(x_bf[i][:], x_nat[i][:])
                xT = xtp.tile([P, 8, 512], bf16, tag="xT")
                for ec in range(8):
                    pst = ps_big.tile([P, 512], bf16, tag="bigT")
                    for i in range(4):
                        nc.tensor.transpose(
                            pst[:, i * 128 : (i + 1) * 128],
                            x_bf[i][:, ec * 128 : (ec + 1) * 128],
                            identb[:],
                        )
                    if ec < 6:
                        nc.vector.tensor_copy(xT[:, ec, :], pst[:])
                    else:
                        nc.scalar.copy(xT[:, ec, :], pst[:])

                # fused (k|v) projection for all 512 rows
                pkv = ps_kv.tile([P, 512], f32, tag="kv")
                for ec in range(8):
                    nc.tensor.matmul(
                        pkv[:],
                        wkv_sb[:, ec, :],
                        xT[:, ec, :],
                        start=(ec == 0),
                        stop=(ec == 7),
                    )
                nc.vector.tensor_copy(kT_sb[:, r0 : r0 + 512], pkv[0:64, :])
                vt_sb = work.tile([64, 512], bf16, tag="vt")
                nc.vector.tensor_copy(vt_sb[:], pkv[64:128, :])
                pvt = ps_small.tile([P, 256], bf16, tag="small")
                for i in range(4):
                    nc.tensor.transpose(
                        pvt[:, i * 64 : (i + 1) * 64],
                        vt_sb[:, i * 128 : (i + 1) * 128],
                        identb[:64, :64],
                    )
                nc.vector.tensor_copy(
                    vOnes[:, 4 * it : 4 * it + 4, 0:64],
                    pvt[:].rearrange("p (b d) -> p b d", b=4),
                )

                # q projection for the two own (even-position) blocks
                pq = ps_kv.tile([64, 256], f32, tag="kv")
                for ec in range(8):
                    rhs = xT[:, ec, :].rearrange(
                        "p (l two c) -> p two l c", l=2, two=2, c=128
                    )[:, 0]
                    nc.tensor.matmul(
                        pq[:], wq_sb[:, ec, :], rhs, start=(ec == 0), stop=(ec == 7)
                    )
                nc.vector.tensor_copy(qT_sb[:, it * 256 : (it + 1) * 256], pq[:])

            # ---- phase 2: segment-based attention ----
            # o_acc[s] accumulates [o | sums] for superblock s in SBUF across
            # kv segments (psum cannot be held open for the whole kernel)
            o_acc = [
                constp.tile([P, 512], f32r, tag=f"oacc{s}", name=f"oacc{s}")
                for s in range(NSUP)
            ]
            seg_first = [True] * NSUP

            def attend_segment(s, kb0, kb1):
                """superblock s attends kv blocks [kb0, kb1)."""
                qT_s = qT_sb[:, s * 512 : (s + 1) * 512]
                po = ps_o.tile([65, 512], f32, tag="po")
                for kb in range(kb0, kb1):
                    k = kb - 8 * s
                    # suffix blocks only reach q column groups t >= k//2
                    c0 = (k // 2) * 128 if k >= 0 else 0
                    pss = ps_big.tile([P, 512], f32, tag="big512")
                    nc.tensor.matmul(
                        pss[:, c0:],
                        kT_sb[:, kb * 128 : (kb + 1) * 128],
                        qT_s[:, c0:],
                        start=True,
                        stop=True,
                    )
                    expT = work.tile([P, 512], bf16, tag="expT")
                    nc.scalar.activation(
                        expT[:, c0:], pss[:, c0:], mybir.ActivationFunctionType.Exp
                    )
                    if k >= 0:
                        # single boundary group: tri (k even) / zeros-or-ones (k odd)
                        nc.vector.tensor_tensor(
                            expT[:, c0 : c0 + 128],
                            expT[:, c0 : c0 + 128],
                            mask_sb[:, k, :],
                            mybir.AluOpType.mult,
                        )
                    nc.tensor.matmul(
                        po[:, c0:],
                        vOnes[:, kb, :],
                        expT[:, c0:],
                        start=(kb == kb0),
                        stop=(kb == kb1 - 1),
                    )
                if seg_first[s]:
                    nc.vector.tensor_copy(o_acc[s][0:65, :], po[:])
                    seg_first[s] = False
                else:
                    nc.vector.tensor_tensor(
                        o_acc[s][0:65, :], o_acc[s][0:65, :], po[:], mybir.AluOpType.add
                    )

            def finish_sup(s):
                # transpose [o | sums] back to q-on-partitions (full 128-wide
                # blocks; rows 65:128 are padding), normalize, store
                o_sb = work.tile([P, 4, 64], f32r, tag="o")
                for th in range(2):
                    pot = ps_small.tile([P, 2, P], f32r, tag="small")
                    for t2 in range(2):
                        t = 2 * th + t2
                        nc.tensor.transpose(
                            pot[:, t2, :],
                            o_acc[s][:, t * 128 : (t + 1) * 128],
                            ident[:],
                        )
                    rec = work.tile([P, 2, 1], f32, tag="recip")
                    nc.vector.reciprocal(rec[:], pot[:, :, 64:65])
                    for t2 in range(2):
                        nc.vector.tensor_scalar_mul(
                            o_sb[:, 2 * th + t2, :], pot[:, t2, 0:64], rec[:, t2]
                        )
                nc.sync.dma_start(
                    y_d.ap()[s * 512 : (s + 1) * 512].rearrange(
                        "(t tt) d -> tt t d", tt=P
                    ),
                    o_sb[:],
                )

            # process x iterations so that late superblocks (long kv spans)
            # get their q early and attend kv segments as they are built;
            # the tail after the last iter shrinks to ~20 kv blocks
            order = [6, 7, 2, 3, 4, 5, 0, 1]
            prefetch_x(order[0])
            load_consts()
            prefetch_x(order[1])
            avail = set()
            done_kv = [set() for _ in range(NSUP)]
            processed = set()
            for jj, j in enumerate(order):
                if jj + 2 < len(order):
                    prefetch_x(order[jj + 2])
                phase1_iter(j)
                processed.add(j)
                avail |= {4 * j + i for i in range(4)}
                for s in range(NSUP):
                    if not (2 * s in processed and 2 * s + 1 in processed):
                        continue
                    span = set(range(8 * (s + 1)))
                    new_kv = sorted((avail & span) - done_kv[s])
                    # contiguous runs
                    run = []
                    for kb in new_kv + [None]:
                        if run and (kb is None or kb != run[-1] + 1):
                            attend_segment(s, run[0], run[-1] + 1)
                            run = []
                        if kb is not None:
                            run.append(kb)
                    done_kv[s] |= set(new_kv)
                    if done_kv[s] == span:
                        finish_sup(s)

    nc.compile()
    return nc


def _host_inputs(x, Wq, Wk, Wv):
    """Build the per-core in_maps (numpy only)."""
    import ml_dtypes

    bf = ml_dtypes.bfloat16
    wkv = np.concatenate([Wk.T, Wv.T], axis=1)  # [E, 128]
    wkv = np.ascontiguousarray(
        wkv.reshape(8, 128, 128).transpose(1, 0, 2).reshape(128, 8 * 128)
    ).astype(bf)
    wq = (Wq.T / np.sqrt(np.float32(D))).astype(np.float32)  # [E, 64], scale folded
    wq = np.ascontiguousarray(
        wq.reshape(8, 128, 64).transpose(1, 0, 2).reshape(128, 8 * 64)
    ).astype(bf)

    tri = np.triu(np.ones((P, P), np.float32))  # keep kv_row tt <= q_row qq
    masks = []
    for p in range(2):
        m = np.zeros((8, P, P), np.float32)
        for k in range(8):
            if k % 2 == 0:
                m[k] = tri
            elif p == 1:
                m[k] = 1.0
        masks.append(
            np.ascontiguousarray(m.transpose(1, 0, 2).reshape(P, 8 * P)).astype(bf)
        )

    swap = np.arange(NBLK).reshape(-1, 2)[:, ::-1].reshape(-1)  # [1,0,3,2,...]
    in_maps = []
    for core in range(8):
        b, p = core // 2, core % 2
        xb = x[b]
        if p == 1:
            xb = xb.reshape(NBLK, P, E)[swap].reshape(S, E)
        in_maps.append(
            {
                "x": np.ascontiguousarray(xb, dtype=np.float32),
                "wkv": wkv,
                "wq": wq,
                "mask": masks[p],
                "ident": np.eye(P, dtype=np.float32),
                "identb": np.eye(P, dtype=np.float32).astype(bf),
                "ones": np.ones((P, NBLK), bf),
            }
        )
    return in_maps


def _assemble(results):
    out = np.empty((B, S, D), np.float32)
    for core in range(8):
        b, p = core // 2, core % 2
        y = np.asarray(results[core]["y"], dtype=np.float32).reshape(16, P, D)
        for j in range(16):
            g = 2 * j + p
            out[b, g * P : (g + 1) * P, :] = y[j]
    return out


def _get_program():
    if "nc" not in _prog_cache:
        _prog_cache["nc"] = _build_program()
    return _prog_cache["nc"]


def run(inputs, trace=False, trace_kwargs=None):
    from concourse import bass_utils

    nc = _get_program()
    in_maps = _host_inputs(
        inputs["x"], inputs["Wq"], inputs["Wk"], inputs["Wv"]
    )
    res = bass_utils.run_bass_kernel_spmd(
        nc,
        in_maps,
        core_ids=list(range(8)),
        trace=trace,
        **(trace_kwargs or {}),
    )
    return _assemble(res.results), res


def kernel(x, Wq, Wk, Wv):
    out, _ = run({"x": x, "Wq": Wq, "Wk": Wk, "Wv": Wv})
    return out


# revision 17
# speedup vs baseline: 1.3192x; 1.3192x over previous
"""Masked causal self-attention on 8 trn2 NeuronCores.

Problem: x[4,4096,1024] fp32; q/k/v = x @ W{q,k,v}.T (D=64);
out = softmax(causal(q k^T / 8)) v   -> [4, 4096, 64].

Sharding: core = (batch, parity). Each core loads its batch's full x,
builds k/v for all 4096 rows, and computes attention for the 2048 q rows
it owns (alternating 128-row blocks by parity). SPMD requires one
program for all cores, so per-core differences are carried by data only:
  - parity-1 cores receive x with adjacent 128-row blocks swapped, so
    every core's own q-blocks sit at even block positions;
  - the causal masks (which differ under that permutation) are inputs.

On-chip dataflow per core (all matmuls float32r = full PE rate):
  x [rows,E] --PE transpose--> xT [E,rows] --matmul--> kT/vT/qT
  scores are computed transposed: S^T[kv,q] = kT-block.T @ qT
  softmax without max-subtraction (scores ~ N(0,1), exp is safe in fp32),
  masked after exp by multiplying with 0/1 mask tiles; the softmax
  denominators come free from an appended ones-column in the V stationary
  ([v | 1] -> row 64 of the output accumulator is sum(exp)).
  oT accumulates in PSUM over kv blocks, is normalized, transposed back,
  and DMA'd out.
"""

import sys

sys.path.insert(0, "/opt/trn_rl_repo")

import numpy as np

B, S, E, D = 4, 4096, 1024, 64
P = 128
NBLK = S // P            # 32 kv block positions
NITER = 8                # phase-1 iterations, 512 rows each
NSUP = 4                 # phase-2 q superblocks, 512 own q rows each
OWN = S // 2             # own q rows per core

_prog_cache = {}


def _build_program():
    import concourse.mybir as mybir
    from concourse import bacc, tile

    f32r = mybir.dt.float32r
    f32 = mybir.dt.float32
    bf16 = mybir.dt.bfloat16

    nc = bacc.Bacc("TRN2", target_bir_lowering=False, debug=False, num_devices=8)
    x_d = nc.dram_tensor("x", [S, E], f32r, kind="ExternalInput")
    wkv_d = nc.dram_tensor("wkv", [P, 8 * 128], bf16, kind="ExternalInput")
    wq_d = nc.dram_tensor("wq", [P, 8 * 64], bf16, kind="ExternalInput")
    mask_d = nc.dram_tensor("mask", [P, 8 * 128], bf16, kind="ExternalInput")
    ident_d = nc.dram_tensor("ident", [P, P], f32r, kind="ExternalInput")
    identb_d = nc.dram_tensor("identb", [P, P], bf16, kind="ExternalInput")
    ones_d = nc.dram_tensor("ones", [P, NBLK], bf16, kind="ExternalInput")
    y_d = nc.dram_tensor("y", [OWN, D], f32r, kind="ExternalOutput")

    with tile.TileContext(nc) as tc:
        with (
            tc.tile_pool(name="const", bufs=1) as constp,
            tc.tile_pool(name="xin", bufs=3) as xin,
            tc.tile_pool(name="xt", bufs=2) as xtp,
            tc.tile_pool(name="work", bufs=3) as work,
            tc.tile_pool(name="ps_big", bufs=2, space="PSUM") as ps_big,
            tc.tile_pool(name="ps_kv", bufs=1, space="PSUM") as ps_kv,
            tc.tile_pool(name="ps_small", bufs=1, space="PSUM") as ps_small,
                        tc.tile_pool(name="ps_o", bufs=2, space="PSUM") as ps_o,
        ):
            # ---- persistent state; constant DMAs are emitted inside the
            # driver loop after the first x prefetch (sync queue) or routed
            # through the idle gpsimd queue ----
            ident = constp.tile([P, P], f32r, tag="ident")
            identb = constp.tile([P, P], bf16, tag="identb")
            wkv_sb = constp.tile([P, 8, 128], bf16, tag="wkv")
            wq_sb = constp.tile([P, 8, 64], bf16, tag="wq")
            mask_sb = constp.tile([P, 8, 128], bf16, tag="mask")
            kT_sb = constp.tile([64, S], bf16, tag="kT")
            qT_sb = constp.tile([64, OWN], bf16, tag="qT")
            vOnes = constp.tile([P, NBLK, 65], bf16, tag="vOnes")

            def load_consts():
                nc.sync.dma_start(identb[:], identb_d.ap())
                nc.sync.dma_start(
                    wkv_sb[:], wkv_d.ap().rearrange("p (c m) -> p c m", c=8)
                )
                nc.sync.dma_start(
                    wq_sb[:], wq_d.ap().rearrange("p (c m) -> p c m", c=8)
                )
                nc.sync.dma_start(
                    mask_sb[:], mask_d.ap().rearrange("p (k c) -> p k c", k=8)
                )
                nc.sync.dma_start(vOnes[:, :, 64], ones_d.ap())
                nc.sync.dma_start(ident[:], ident_d.ap())

            # ---- phase 1: prefetch (DMA) and compute bodies ----
            x_tiles = {}

            def prefetch_x(it):
                r0 = it * 512
                blks = []
                for i in range(4):
                    xn = xin.tile([P, E], f32r, tag=f"xnat{i}", name=f"xnat_{it}_{i}")
                    nc.sync.dma_start(
                        xn[:],
                        x_d.ap()[r0 + i * P : r0 + (i + 1) * P].rearrange(
                            "(i p) e -> p (i e)", i=1
                        ),
                    )
                    blks.append(xn)
                x_tiles[it] = blks

            def phase1_iter(it):
                r0 = it * 512
                x_nat = x_tiles.pop(it)
                x_bf = [
                    xin.tile([P, E], bf16, tag=f"xbf{i}", name=f"xbf_{it}_{i}")
                    for i in range(4)
                ]
                for i in range(4):
                    if i < 2:
                        nc.vector.tensor_copy(x_bf[i][:], x_nat[i][:])
                    else:
                        nc.scalar.copy(x_bf[i][:], x_nat[i][:])
                xT = xtp.tile([P, 8, 512], bf16, tag="xT")
                for ec in range(8):
                    pst = ps_big.tile([P, 512], bf16, tag="bigT")
                    for i in range(4):
                        nc.tensor.transpose(
                            pst[:, i * 128 : (i + 1) * 128],
                            x_bf[i][:, ec * 128 : (ec + 1) * 128],
                            identb[:],
                        )
                    if ec < 6:
                        nc.vector.tensor_copy(xT[:, ec, :], pst[:])
                    else:
                        nc.scalar.copy(xT[:, ec, :], pst[:])

                # fused (k|v) projection for all 512 rows
                pkv = ps_kv.tile([P, 512], f32, tag="kv")
                for ec in range(8):
                    nc.tensor.matmul(
                        pkv[:],
                        wkv_sb[:, ec, :],
                        xT[:, ec, :],
                        start=(ec == 0),
                        stop=(ec == 7),
                    )
                nc.vector.tensor_copy(kT_sb[:, r0 : r0 + 512], pkv[0:64, :])
                vt_sb = work.tile([64, 512], bf16, tag="vt")
                nc.vector.tensor_copy(vt_sb[:], pkv[64:128, :])
                pvt = ps_small.tile([P, 256], bf16, tag="small")
                for i in range(4):
                    nc.tensor.transpose(
                        pvt[:, i * 64 : (i + 1) * 64],
                        vt_sb[:, i * 128 : (i + 1) * 128],
                        identb[:64, :64],
                    )
                nc.vector.tensor_copy(
                    vOnes[:, 4 * it : 4 * it + 4, 0:64],
                    pvt[:].rearrange("p (b d) -> p b d", b=4),
                )

                # q projection for the two own (even-position) blocks
                pq = ps_kv.tile([64, 256], f32, tag="kv")
                for ec in range(8):
                    rhs = xT[:, ec, :].rearrange(
                        "p (l two c) -> p two l c", l=2, two=2, c=128
                    )[:, 0]
                    nc.tensor.matmul(
                        pq[:], wq_sb[:, ec, :], rhs, start=(ec == 0), stop=(ec == 7)
                    )
                nc.vector.tensor_copy(qT_sb[:, it * 256 : (it + 1) * 256], pq[:])

            # ---- phase 2: segment-based attention ----
            # o_acc[s] accumulates [o | sums] for superblock s in SBUF across
            # kv segments (psum cannot be held open for the whole kernel)
            o_acc = [
                constp.tile([P, 512], f32r, tag=f"oacc{s}", name=f"oacc{s}")
                for s in range(NSUP)
            ]
            seg_first = [True] * NSUP

            def attend_segment(s, kb0, kb1):
                """superblock s attends kv blocks [kb0, kb1)."""
                qT_s = qT_sb[:, s * 512 : (s + 1) * 512]
                po = ps_o.tile([65, 512], f32, tag="po")
                for kb in range(kb0, kb1):
                    k = kb - 8 * s
                    # suffix blocks only reach q column groups t >= k//2
                    c0 = (k // 2) * 128 if k >= 0 else 0
                    pss = ps_big.tile([P, 512], f32, tag="big512")
                    nc.tensor.matmul(
                        pss[:, c0:],
                        kT_sb[:, kb * 128 : (kb + 1) * 128],
                        qT_s[:, c0:],
                        start=True,
                        stop=True,
                    )
                    expT = work.tile([P, 512], bf16, tag="expT")
                    nc.scalar.activation(
                        expT[:, c0:], pss[:, c0:], mybir.ActivationFunctionType.Exp
                    )
                    if k >= 0:
                        # single boundary group: tri (k even) / zeros-or-ones (k odd)
                        nc.vector.tensor_tensor(
                            expT[:, c0 : c0 + 128],
                            expT[:, c0 : c0 + 128],
                            mask_sb[:, k, :],
                            mybir.AluOpType.mult,
                        )
                    nc.tensor.matmul(
                        po[:, c0:],
                        vOnes[:, kb, :],
                        expT[:, c0:],
                        start=(kb == kb0),
                        stop=(kb == kb1 - 1),
                    )
                if seg_first[s]:
                    nc.vector.tensor_copy(o_acc[s][0:65, :], po[:])
                    seg_first[s] = False
                else:
                    nc.vector.tensor_tensor(
                        o_acc[s][0:65, :], o_acc[s][0:65, :], po[:], mybir.AluOpType.add
                    )

            def finish_sup(s):
                # transpose [o | sums] back to q-on-partitions (full 128-wide
                # blocks; rows 65:128 are padding), normalize, store
                o_sb = work.tile([P, 4, 64], f32r, tag="o")
                for th in range(2):
                    pot = ps_small.tile([P, 2, P], f32r, tag="small")
                    for t2 in range(2):
                        t = 2 * th + t2
                        nc.tensor.transpose(
                            pot[:, t2, :],
                            o_acc[s][:, t * 128 : (t + 1) * 128],
                            ident[:],
                        )
                    rec = work.tile([P, 2, 1], f32, tag="recip")
                    nc.vector.reciprocal(rec[:], pot[:, :, 64:65])
                    for t2 in range(2):
                        nc.vector.tensor_scalar_mul(
                            o_sb[:, 2 * th + t2, :], pot[:, t2, 0:64], rec[:, t2]
                        )
                nc.sync.dma_start(
                    y_d.ap()[s * 512 : (s + 1) * 512].rearrange(
                        "(t tt) d -> tt t d", tt=P
                    ),
                    o_sb[:],
                )

            # process x iterations so that late superblocks (long kv spans)
            # get their q early and attend kv segments as they are built;
            # the tail after the last iter shrinks to ~20 kv blocks
            order = [6, 7, 2, 3, 4, 5, 0, 1]
            prefetch_x(order[0])
            load_consts()
            prefetch_x(order[1])
            avail = set()
            done_kv = [set() for _ in range(NSUP)]
            processed = set()
            for jj, j in enumerate(order):
                if jj + 2 < len(order):
                    prefetch_x(order[jj + 2])
                phase1_iter(j)
                processed.add(j)
                avail |= {4 * j + i for i in range(4)}
                for s in range(NSUP):
                    if not (2 * s in processed and 2 * s + 1 in processed):
                        continue
                    span = set(range(8 * (s + 1)))
                    new_kv = sorted((avail & span) - done_kv[s])
                    # contiguous runs
                    run = []
                    for kb in new_kv + [None]:
                        if run and (kb is None or kb != run[-1] + 1):
                            attend_segment(s, run[0], run[-1] + 1)
                            run = []
                        if kb is not None:
                            run.append(kb)
                    done_kv[s] |= set(new_kv)
                    if done_kv[s] == span:
                        finish_sup(s)

    nc.compile()
    return nc


def _host_inputs(x, Wq, Wk, Wv):
    """Build the per-core in_maps (numpy only)."""
    import ml_dtypes

    bf = ml_dtypes.bfloat16
    wkv = np.concatenate([Wk.T, Wv.T], axis=1)  # [E, 128]
    wkv = np.ascontiguousarray(
        wkv.reshape(8, 128, 128).transpose(1, 0, 2).reshape(128, 8 * 128)
    ).astype(bf)
    wq = (Wq.T / np.sqrt(np.float32(D))).astype(np.float32)  # [E, 64], scale folded
    wq = np.ascontiguousarray(
        wq.reshape(8, 128, 64).transpose(1, 0, 2).reshape(128, 8 * 64)
    ).astype(bf)

    tri = np.triu(np.ones((P, P), np.float32))  # keep kv_row tt <= q_row qq
    masks = []
    for p in range(2):
        m = np.zeros((8, P, P), np.float32)
        for k in range(8):
            if k % 2 == 0:
                m[k] = tri
            elif p == 1:
                m[k] = 1.0
        masks.append(
            np.ascontiguousarray(m.transpose(1, 0, 2).reshape(P, 8 * P)).astype(bf)
        )

    swap = np.arange(NBLK).reshape(-1, 2)[:, ::-1].reshape(-1)  # [1,0,3,2,...]
    in_maps = []
    for core in range(8):
        b, p = core // 2, core % 2
        xb = x[b]
        if p == 1:
            xb = xb.reshape(NBLK, P, E)[swap].reshape(S, E)
        in_maps.append(
            {
                "x": np.ascontiguousarray(xb, dtype=np.float32),
                "wkv": wkv,
                "wq": wq,
                "mask": masks[p],
                "ident": np.eye(P, dtype=np.float32),
                "identb": np.eye(P, dtype=np.float32).astype(bf),
                "ones": np.ones((P, NBLK), bf),
            }
        )
    return in_maps


def _assemble(results):
    out = np.empty((B, S, D), np.float32)
    for core in range(8):
        b, p = core // 2, core % 2
        y = np.asarray(results[core]["y"], dtype=np.float32).reshape(16, P, D)
        for j in range(16):
            g = 2 * j + p
            out[b, g * P : (g + 1) * P, :] = y[j]
    return out


def _get_program():
    if "nc" not in _prog_cache:
        _prog_cache["nc"] = _build_program()
    return _prog_cache["nc"]


def run(inputs, trace=False, trace_kwargs=None):
    from concourse import bass_utils

    nc = _get_program()
    in_maps = _host_inputs(
        inputs["x"], inputs["Wq"], inputs["Wk"], inputs["Wv"]
    )
    res = bass_utils.run_bass_kernel_spmd(
        nc,
        in_maps,
        core_ids=list(range(8)),
        trace=trace,
        **(trace_kwargs or {}),
    )
    return _assemble(res.results), res


def kernel(x, Wq, Wk, Wv):
    out, _ = run({"x": x, "Wq": Wq, "Wk": Wk, "Wv": Wv})
    return out


# revision 21
# speedup vs baseline: 1.3578x; 1.0293x over previous
"""Masked causal self-attention on 8 trn2 NeuronCores.

Problem: x[4,4096,1024] fp32; q/k/v = x @ W{q,k,v}.T (D=64);
out = softmax(causal(q k^T / 8)) v   -> [4, 4096, 64].

Sharding: core = (batch, parity). Each core loads its batch's full x,
builds k/v for all 4096 rows, and computes attention for the 2048 q rows
it owns (alternating 128-row blocks by parity). SPMD requires one
program for all cores, so per-core differences are carried by data only:
  - parity-1 cores receive x with adjacent 128-row blocks swapped, so
    every core's own q-blocks sit at even block positions;
  - the causal masks (which differ under that permutation) are inputs.

On-chip dataflow per core (all matmuls float32r = full PE rate):
  x [rows,E] --PE transpose--> xT [E,rows] --matmul--> kT/vT/qT
  scores are computed transposed: S^T[kv,q] = kT-block.T @ qT
  softmax without max-subtraction (scores ~ N(0,1), exp is safe in fp32),
  masked after exp by multiplying with 0/1 mask tiles; the softmax
  denominators come free from an appended ones-column in the V stationary
  ([v | 1] -> row 64 of the output accumulator is sum(exp)).
  oT accumulates in PSUM over kv blocks, is normalized, transposed back,
  and DMA'd out.
"""

import sys

sys.path.insert(0, "/opt/trn_rl_repo")

import numpy as np

B, S, E, D = 4, 4096, 1024, 64
P = 128
NBLK = S // P            # 32 kv block positions
NITER = 8                # phase-1 iterations, 512 rows each
NSUP = 4                 # phase-2 q superblocks, 512 own q rows each
OWN = S // 2             # own q rows per core

_prog_cache = {}


def _build_program():
    import concourse.mybir as mybir
    from concourse import bacc, tile

    f32r = mybir.dt.float32r
    f32 = mybir.dt.float32
    bf16 = mybir.dt.bfloat16

    nc = bacc.Bacc("TRN2", target_bir_lowering=False, debug=False, num_devices=8)
    x_d = nc.dram_tensor("x", [S, E], f32r, kind="ExternalInput")
    wkv_d = nc.dram_tensor("wkv", [P, 8 * 128], bf16, kind="ExternalInput")
    wq_d = nc.dram_tensor("wq", [P, 8 * 64], bf16, kind="ExternalInput")
    mask_d = nc.dram_tensor("mask", [P, 8 * 128], bf16, kind="ExternalInput")
    ident_d = nc.dram_tensor("ident", [P, P], f32r, kind="ExternalInput")
    identb_d = nc.dram_tensor("identb", [P, P], bf16, kind="ExternalInput")
    ones_d = nc.dram_tensor("ones", [P, NBLK], bf16, kind="ExternalInput")
    y_d = nc.dram_tensor("y", [OWN, D], f32r, kind="ExternalOutput")

    with tile.TileContext(nc) as tc:
        with (
            tc.tile_pool(name="const", bufs=1) as constp,
            tc.tile_pool(name="xin", bufs=3) as xin,
            tc.tile_pool(name="xt", bufs=2) as xtp,
            tc.tile_pool(name="work", bufs=3) as work,
            tc.tile_pool(name="ps_big", bufs=2, space="PSUM") as ps_big,
            tc.tile_pool(name="ps_kv", bufs=1, space="PSUM") as ps_kv,
                                    tc.tile_pool(name="ps_o", bufs=1, space="PSUM") as ps_o,
        ):
            # ---- persistent state; constant DMAs are emitted inside the
            # driver loop after the first x prefetch (sync queue) or routed
            # through the idle gpsimd queue ----
            ident = constp.tile([P, P], f32r, tag="ident")
            identb = constp.tile([P, P], bf16, tag="identb")
            wkv_sb = constp.tile([P, 8, 128], bf16, tag="wkv")
            wq_sb = constp.tile([P, 8, 64], bf16, tag="wq")
            mask_sb = constp.tile([P, 8, 128], bf16, tag="mask")
            kT_sb = constp.tile([64, S], bf16, tag="kT")
            qT_sb = constp.tile([64, OWN], bf16, tag="qT")
            vOnes = constp.tile([P, NBLK, 65], bf16, tag="vOnes")

            def load_consts():
                nc.sync.dma_start(identb[:], identb_d.ap())
                nc.sync.dma_start(
                    wkv_sb[:], wkv_d.ap().rearrange("p (c m) -> p c m", c=8)
                )
                nc.sync.dma_start(
                    wq_sb[:], wq_d.ap().rearrange("p (c m) -> p c m", c=8)
                )
                nc.sync.dma_start(
                    mask_sb[:], mask_d.ap().rearrange("p (k c) -> p k c", k=8)
                )
                nc.sync.dma_start(vOnes[:, :, 64], ones_d.ap())
                nc.sync.dma_start(ident[:], ident_d.ap())

            # ---- phase 1: prefetch (DMA) and compute bodies ----
            x_tiles = {}

            def prefetch_x(it):
                r0 = it * 512
                blks = []
                for i in range(4):
                    xn = xin.tile([P, E], f32r, tag=f"xnat{i}", name=f"xnat_{it}_{i}")
                    nc.sync.dma_start(
                        xn[:],
                        x_d.ap()[r0 + i * P : r0 + (i + 1) * P].rearrange(
                            "(i p) e -> p (i e)", i=1
                        ),
                    )
                    blks.append(xn)
                x_tiles[it] = blks

            def phase1_iter(it):
                r0 = it * 512
                x_nat = x_tiles.pop(it)
                x_bf = [
                    xin.tile([P, E], bf16, tag=f"xbf{i}", name=f"xbf_{it}_{i}")
                    for i in range(4)
                ]
                for i in range(4):
                    if i < 2:
                        nc.vector.tensor_copy(x_bf[i][:], x_nat[i][:])
                    else:
                        nc.scalar.copy(x_bf[i][:], x_nat[i][:])
                xT = [
                    xtp.tile([P, 512], bf16, tag=f"xT{ec}", name=f"xT_{it}_{ec}")
                    for ec in range(8)
                ]
                for ec in range(8):
                    pst = ps_big.tile([P, 512], bf16, tag="bigT")
                    for i in range(4):
                        nc.tensor.transpose(
                            pst[:, i * 128 : (i + 1) * 128],
                            x_bf[i][:, ec * 128 : (ec + 1) * 128],
                            identb[:],
                        )
                    if ec < 6:
                        nc.vector.tensor_copy(xT[ec][:], pst[:])
                    else:
                        nc.scalar.copy(xT[ec][:], pst[:])

                # fused (k|v) projection for all 512 rows
                pkv = ps_kv.tile([P, 512], f32, tag="kv")
                for ec in range(8):
                    nc.tensor.matmul(
                        pkv[:],
                        wkv_sb[:, ec, :],
                        xT[ec][:],
                        start=(ec == 0),
                        stop=(ec == 7),
                    )
                nc.vector.tensor_copy(kT_sb[:, r0 : r0 + 512], pkv[0:64, :])
                vt_sb = work.tile([64, 512], bf16, tag="vt")
                nc.vector.tensor_copy(vt_sb[:], pkv[64:128, :])
                pvt = ps_kv.tile([P, 256], bf16, tag="kv")
                for i in range(4):
                    nc.tensor.transpose(
                        pvt[:, i * 64 : (i + 1) * 64],
                        vt_sb[:, i * 128 : (i + 1) * 128],
                        identb[:64, :64],
                    )
                nc.vector.tensor_copy(
                    vOnes[:, 4 * it : 4 * it + 4, 0:64],
                    pvt[:].rearrange("p (b d) -> p b d", b=4),
                )

                # q projection for the two own (even-position) blocks
                pq = ps_kv.tile([64, 256], f32, tag="kv")
                for ec in range(8):
                    rhs = xT[ec][:].rearrange(
                        "p (l two c) -> p two l c", l=2, two=2, c=128
                    )[:, 0]
                    nc.tensor.matmul(
                        pq[:], wq_sb[:, ec, :], rhs, start=(ec == 0), stop=(ec == 7)
                    )
                nc.vector.tensor_copy(qT_sb[:, it * 256 : (it + 1) * 256], pq[:])

            # ---- phase 2: segment-based attention ----
            # o_acc[s] accumulates [o | sums] for superblock s in SBUF across
            # kv segments (psum cannot be held open for the whole kernel)
            o_acc = [
                constp.tile([P, 512], f32r, tag=f"oacc{s}", name=f"oacc{s}")
                for s in range(NSUP)
            ]
            seg_first = [True] * NSUP

            def attend_segment(s, kb0, kb1):
                """superblock s attends kv blocks [kb0, kb1), two at a time:
                one [128, 2, 512] psum pair -> one exp -> two AV matmuls."""
                assert (kb1 - kb0) % 2 == 0 and kb0 % 2 == 0
                qT_s = qT_sb[:, s * 512 : (s + 1) * 512]
                po = ps_o.tile([65, 512], f32, tag="po")
                for pb in range(kb0, kb1, 2):
                    k = pb - 8 * s
                    # suffix pairs only reach q column groups t >= k//2
                    c0 = (k // 2) * 128 if k >= 0 else 0
                    ps2 = ps_big.tile([P, 2, 512], f32, tag="big1024")
                    for j in range(2):
                        nc.tensor.matmul(
                            ps2[:, j, c0:],
                            kT_sb[:, (pb + j) * 128 : (pb + j + 1) * 128],
                            qT_s[:, c0:],
                            start=True,
                            stop=True,
                        )
                    expT = work.tile([P, 2, 512], bf16, tag="expT")
                    nc.scalar.activation(
                        expT[:, :, c0:], ps2[:, :, c0:],
                        mybir.ActivationFunctionType.Exp,
                    )
                    if k >= 0:
                        # boundary group: tri (even k) / zeros-or-ones (odd k)
                        for j in range(2):
                            nc.vector.tensor_tensor(
                                expT[:, j, c0 : c0 + 128],
                                expT[:, j, c0 : c0 + 128],
                                mask_sb[:, k + j, :],
                                mybir.AluOpType.mult,
                            )
                    for j in range(2):
                        nc.tensor.matmul(
                            po[:, c0:],
                            vOnes[:, pb + j, :],
                            expT[:, j, c0:],
                            start=(pb == kb0 and j == 0),
                            stop=(pb + j == kb1 - 1),
                        )
                if seg_first[s]:
                    nc.vector.tensor_copy(o_acc[s][0:65, :], po[:])
                    seg_first[s] = False
                else:
                    nc.vector.tensor_tensor(
                        o_acc[s][0:65, :], o_acc[s][0:65, :], po[:], mybir.AluOpType.add
                    )

            def finish_sup(s):
                # transpose [o | sums] back to q-on-partitions (full 128-wide
                # blocks; rows 65:128 are padding), normalize, store
                o_sb = work.tile([P, 4, 64], f32r, tag="o")
                for th in range(2):
                    pot = ps_kv.tile([P, 2, P], f32r, tag="kv")
                    for t2 in range(2):
                        t = 2 * th + t2
                        nc.tensor.transpose(
                            pot[:, t2, :],
                            o_acc[s][:, t * 128 : (t + 1) * 128],
                            ident[:],
                        )
                    rec = work.tile([P, 2, 1], f32, tag="recip")
                    nc.vector.reciprocal(rec[:], pot[:, :, 64:65])
                    for t2 in range(2):
                        nc.vector.tensor_scalar_mul(
                            o_sb[:, 2 * th + t2, :], pot[:, t2, 0:64], rec[:, t2]
                        )
                nc.sync.dma_start(
                    y_d.ap()[s * 512 : (s + 1) * 512].rearrange(
                        "(t tt) d -> tt t d", tt=P
                    ),
                    o_sb[:],
                )

            # process x iterations so that late superblocks (long kv spans)
            # get their q early and attend kv segments as they are built;
            # the tail after the last iter shrinks to ~20 kv blocks
            order = [6, 7, 2, 3, 4, 5, 0, 1]
            prefetch_x(order[0])
            load_consts()
            prefetch_x(order[1])
            avail = set()
            done_kv = [set() for _ in range(NSUP)]
            processed = set()
            for jj, j in enumerate(order):
                if jj + 2 < len(order):
                    prefetch_x(order[jj + 2])
                phase1_iter(j)
                processed.add(j)
                avail |= {4 * j + i for i in range(4)}
                for s in range(NSUP):
                    if not (2 * s in processed and 2 * s + 1 in processed):
                        continue
                    span = set(range(8 * (s + 1)))
                    new_kv = sorted((avail & span) - done_kv[s])
                    # contiguous runs
                    run = []
                    for kb in new_kv + [None]:
                        if run and (kb is None or kb != run[-1] + 1):
                            attend_segment(s, run[0], run[-1] + 1)
                            run = []
                        if kb is not None:
                            run.append(kb)
                    done_kv[s] |= set(new_kv)
                    if done_kv[s] == span:
                        finish_sup(s)

    nc.compile()
    return nc


def _host_inputs(x, Wq, Wk, Wv):
    """Build the per-core in_maps (numpy only)."""
    import ml_dtypes

    bf = ml_dtypes.bfloat16
    wkv = np.concatenate([Wk.T, Wv.T], axis=1)  # [E, 128]
    wkv = np.ascontiguousarray(
        wkv.reshape(8, 128, 128).transpose(1, 0, 2).reshape(128, 8 * 128)
    ).astype(bf)
    wq = (Wq.T / np.sqrt(np.float32(D))).astype(np.float32)  # [E, 64], scale folded
    wq = np.ascontiguousarray(
        wq.reshape(8, 128, 64).transpose(1, 0, 2).reshape(128, 8 * 64)
    ).astype(bf)

    tri = np.triu(np.ones((P, P), np.float32))  # keep kv_row tt <= q_row qq
    masks = []
    for p in range(2):
        m = np.zeros((8, P, P), np.float32)
        for k in range(8):
            if k % 2 == 0:
                m[k] = tri
            elif p == 1:
                m[k] = 1.0
        masks.append(
            np.ascontiguousarray(m.transpose(1, 0, 2).reshape(P, 8 * P)).astype(bf)
        )

    swap = np.arange(NBLK).reshape(-1, 2)[:, ::-1].reshape(-1)  # [1,0,3,2,...]
    in_maps = []
    for core in range(8):
        b, p = core // 2, core % 2
        xb = x[b]
        if p == 1:
            xb = xb.reshape(NBLK, P, E)[swap].reshape(S, E)
        in_maps.append(
            {
                "x": np.ascontiguousarray(xb, dtype=np.float32),
                "wkv": wkv,
                "wq": wq,
                "mask": masks[p],
                "ident": np.eye(P, dtype=np.float32),
                "identb": np.eye(P, dtype=np.float32).astype(bf),
                "ones": np.ones((P, NBLK), bf),
            }
        )
    return in_maps


def _assemble(results):
    out = np.empty((B, S, D), np.float32)
    for core in range(8):
        b, p = core // 2, core % 2
        y = np.asarray(results[core]["y"], dtype=np.float32).reshape(16, P, D)
        for j in range(16):
            g = 2 * j + p
            out[b, g * P : (g + 1) * P, :] = y[j]
    return out


def _get_program():
    if "nc" not in _prog_cache:
        _prog_cache["nc"] = _build_program()
    return _prog_cache["nc"]


def run(inputs, trace=False, trace_kwargs=None):
    from concourse import bass_utils

    nc = _get_program()
    in_maps = _host_inputs(
        inputs["x"], inputs["Wq"], inputs["Wk"], inputs["Wv"]
    )
    res = bass_utils.run_bass_kernel_spmd(
        nc,
        in_maps,
        core_ids=list(range(8)),
        trace=trace,
        **(trace_kwargs or {}),
    )
    return _assemble(res.results), res


def kernel(x, Wq, Wk, Wv):
    out, _ = run({"x": x, "Wq": Wq, "Wk": Wk, "Wv": Wv})
    return out
